# revision 4
# baseline (speedup 1.0000x reference)
"""Bahdanau attention kernel for 8 Trainium2 NeuronCores.

Data-parallel over batch B=32 (4 batches per core); Wa/Ua/Va weights
replicated on every core. Per core:
  q  = query @ Wa^T + Wa_b + Ua_b          (tiny, on PE)
  kT = transpose(keys) blocks on PE; k = keys @ Ua^T via fp32r matmuls
  h  = tanh(k + q) fused on ScalarE (per-partition bias)
  s  = Va . h via fp32r matmuls -> softmax (Va_b cancels in softmax)
  ctx = softmax(s) @ keys via fp32 matmuls (keys streamed a 2nd time)
"""
import sys

sys.path.insert(0, "/opt/trn_rl_repo")

import numpy as np

B, S, H = 32, 2048, 1024
NCORES = 8
BPC = B // NCORES   # batches per core
HB = H // 128       # 8 blocks of 128 along H
SB = S // 128       # 16 blocks of 128 along S
SCH = 512           # s-chunk for the Ua matmul / tanh / Va pipeline
NSCH = S // SCH
SG = 512            # s-group per keys DMA during the transpose phase

_cache = {}


def _build_program():
    from contextlib import ExitStack

    import concourse.bass as bass  # noqa: F401
    from concourse import bacc, mybir
    from concourse.masks import make_identity
    from concourse.tile import TileContext

    F32 = mybir.dt.float32
    F32R = mybir.dt.float32r
    Act = mybir.ActivationFunctionType

    nc = bacc.Bacc()
    keys_d = nc.declare_dram_parameter("keys", [BPC, S, H], F32, isOutput=False)
    query_d = nc.declare_dram_parameter("query", [BPC, H], F32, isOutput=False)
    wa_d = nc.declare_dram_parameter("wa", [H, H], F32, isOutput=False)
    ua_d = nc.declare_dram_parameter("ua", [H, H], F32, isOutput=False)
    wab_d = nc.declare_dram_parameter("wab", [HB, 128], F32, isOutput=False)
    uab_d = nc.declare_dram_parameter("uab", [HB, 128], F32, isOutput=False)
    va_d = nc.declare_dram_parameter("va", [HB, 128], F32, isOutput=False)
    ctx_d = nc.declare_dram_parameter("ctx", [BPC, H], F32, isOutput=True)
    wout_d = nc.declare_dram_parameter("wout", [BPC, S], F32, isOutput=True)

    with TileContext(nc) as tc, ExitStack() as ex:
        singles = ex.enter_context(tc.tile_pool(name="singles", bufs=1))
        stream = ex.enter_context(tc.tile_pool(name="stream", bufs=3))
        ht_pool = ex.enter_context(tc.tile_pool(name="ht", bufs=3))
        keys1_pool = ex.enter_context(tc.tile_pool(name="keys1p", bufs=2))
        sm_pool = ex.enter_context(tc.tile_pool(name="sm", bufs=1))
        ps_t = ex.enter_context(tc.tile_pool(name="ps_t", bufs=3, space="PSUM"))
        ps_mm = ex.enter_context(tc.tile_pool(name="ps_mm", bufs=2, space="PSUM"))
        ps_sc = ex.enter_context(tc.tile_pool(name="ps_sc", bufs=2, space="PSUM"))

        ident = singles.tile([128, 128], F32)
        make_identity(nc, ident[:, :])

        # biases: bias_sum[p, i] = Wa_b[i*128+p] + Ua_b[i*128+p]
        bias_sum = singles.tile([128, HB], F32)
        tmp_b = singles.tile([128, HB], F32)
        nc.sync.dma_start(out=bias_sum[:, :], in_=wab_d[:, :].rearrange("j p -> p j"))
        nc.sync.dma_start(out=tmp_b[:, :], in_=uab_d[:, :].rearrange("j p -> p j"))
        nc.vector.tensor_add(bias_sum[:, :], bias_sum[:, :], tmp_b[:, :])

        # vaT[p, i] = Va_w[0, i*128+p], rounded to fp32r
        va_f32 = singles.tile([128, HB], F32)
        nc.sync.dma_start(out=va_f32[:, :], in_=va_d[:, :].rearrange("j p -> p j"))
        vaT = singles.tile([128, HB], F32R)
        nc.vector.tensor_copy(vaT[:, :], va_f32[:, :])

        def load_T(w_dram, name, pool):
            """DRAM [H,H] row-major W[o,h] -> SBUF [128, HB, H] fp32r W^T:
            wt[p, j, o] = W[o, j*128+p]."""
            wt = pool.tile([128, HB, H], F32R, tag=name)
            for i in range(HB):  # o-block
                nat = stream.tile([128, H], F32, tag="keys2")
                nc.sync.dma_start(out=nat[:, :], in_=w_dram[i * 128:(i + 1) * 128, :])
                for j4 in range(0, HB, 4):
                    pt = ps_t.tile([128, 512], F32, tag="pt")
                    for q in range(4):
                        j = j4 + q
                        nc.tensor.transpose(
                            pt[:, q * 128:(q + 1) * 128],
                            nat[:, j * 128:(j + 1) * 128],
                            ident[:, :],
                        )
                    dst = wt[:, j4:j4 + 4, i * 128:(i + 1) * 128]
                    src = pt[:, :].rearrange("p (q x) -> p q x", q=4)
                    if (i + j4 // 4) % 2 == 0:
                        nc.vector.tensor_copy(dst, src)
                    else:
                        nc.scalar.activation(out=dst, in_=src, func=Act.Copy, scale=1.0)
            return wt

        # ---- q phase: qq[p, i, b] = (query @ Wa^T)[b, i*128+p] + bias_sum
        waT_cm = tc.tile_pool(name="waTp", bufs=1)
        waTp = waT_cm.__enter__()
        waT = load_T(wa_d, "waT", waTp)
        qsb = singles.tile([BPC, H], F32)
        nc.sync.dma_start(out=qsb[:, :], in_=query_d[:, :])
        qT = singles.tile([128, HB, BPC], F32R)
        for j in range(HB):
            ptq = ps_t.tile([128, 512], F32, tag="pt")
            nc.tensor.transpose(
                ptq[:, 0:BPC], qsb[0:BPC, j * 128:(j + 1) * 128], ident[0:BPC, 0:BPC]
            )
            nc.vector.tensor_copy(qT[:, j, :], ptq[:, 0:BPC])
        qq = singles.tile([128, HB, BPC], F32)
        for i in range(HB):
            pq = ps_mm.tile([128, 512], F32, tag="kpsum")
            for j in range(HB):
                nc.tensor.matmul(
                    pq[:, 0:BPC],
                    waT[:, j, i * 128:(i + 1) * 128],
                    qT[:, j, :],
                    start=(j == 0),
                    stop=(j == HB - 1),
                )
            nc.vector.tensor_scalar_add(qq[:, i, :], pq[:, 0:BPC], bias_sum[:, i:i + 1])

        waT_cm.__exit__(None, None, None)

        uaT = load_T(ua_d, "uaT", singles)
        kt_pool = ex.enter_context(tc.tile_pool(name="kTp", bufs=1))

        # ---- per-batch pipeline
        for b in range(BPC):
            # A: transpose keys into kT[p, j, s] = keys[b, s, j*128+p] (fp32r)
            kT = kt_pool.tile([128, HB, S], F32R, tag="kT")
            for g in range(S // SG):
                kg = keys1_pool.tile([128, SG // 128, H], F32, tag="keys1")
                nc.sync.dma_start(
                    out=kg[:, :, :],
                    in_=keys_d[b, g * SG:(g + 1) * SG, :].rearrange(
                        "(t p) h -> p t h", p=128
                    ),
                )
                for j in range(HB):
                    pt = ps_t.tile([128, 512], F32, tag="pt")
                    for t4 in range(SG // 128):
                        nc.tensor.transpose(
                            pt[:, t4 * 128:(t4 + 1) * 128],
                            kg[:, t4, j * 128:(j + 1) * 128],
                            ident[:, :],
                        )
                    dst = kT[:, j, g * SG:(g + 1) * SG]
                    if j % 2 == 0:
                        nc.vector.tensor_copy(dst, pt[:, :])
                    else:
                        nc.scalar.activation(
                            out=dst, in_=pt[:, :], func=Act.Copy, scale=1.0
                        )

            # B: k = keys @ Ua^T (fp32r), h = tanh(k + qq), s = Va . h
            scores = sm_pool.tile([1, S], F32, tag="scores")
            pending = None  # delay each Va matmul past the next kpsum group
            scps = []
            for c in range(NSCH):
                scp = ps_sc.tile([1, SCH], F32, tag="scps")
                scps.append(scp)
                for i in range(HB):
                    kp = ps_mm.tile([128, SCH], F32, tag="kpsum")
                    for j in range(HB):
                        nc.tensor.matmul(
                            kp[:, :],
                            uaT[:, j, i * 128:(i + 1) * 128],
                            kT[:, j, c * SCH:(c + 1) * SCH],
                            start=(j == 0),
                            stop=(j == HB - 1),
                        )
                    ht = ht_pool.tile([128, SCH], F32R, tag="ht")
                    nc.scalar.activation(
                        out=ht[:, :],
                        in_=kp[:, :],
                        func=Act.Tanh,
                        bias=qq[:, i, b:b + 1],
                        scale=1.0,
                    )
                    if pending is not None:
                        pht, pscp, pi = pending
                        nc.tensor.matmul(
                            pscp[:, :],
                            vaT[:, pi:pi + 1],
                            pht[:, :],
                            start=(pi == 0),
                            stop=(pi == HB - 1),
                        )
                        if pi == HB - 1:
                            pc = scps.index(pscp)
                            nc.vector.tensor_copy(
                                scores[0:1, pc * SCH:(pc + 1) * SCH], pscp[:, :]
                            )
                    pending = (ht, scp, i)
            # flush the last Va matmul
            pht, pscp, pi = pending
            nc.tensor.matmul(
                pscp[:, :], vaT[:, pi:pi + 1], pht[:, :],
                start=(pi == 0), stop=(pi == HB - 1),
            )
            nc.vector.tensor_copy(
                scores[0:1, (NSCH - 1) * SCH:NSCH * SCH], pscp[:, :]
            )

            # C: softmax on partition 0
            nmax = sm_pool.tile([1, 1], F32, tag="nmax")
            nc.vector.tensor_reduce(
                out=nmax[:, :], in_=scores[:, :],
                axis=mybir.AxisListType.X, op=mybir.AluOpType.max, negate=True,
            )
            expv = sm_pool.tile([1, S], F32, tag="expv")
            ssum = sm_pool.tile([1, 1], F32, tag="ssum")
            nc.scalar.activation(
                out=expv[:, :], in_=scores[:, :], func=Act.Exp,
                bias=nmax[:, :], scale=1.0, accum_out=ssum[:, :],
            )
            rz = sm_pool.tile([1, 1], F32, tag="rz")
            nc.vector.reciprocal(rz[:, :], ssum[:, :])
            wnorm = sm_pool.tile([1, S], F32, tag="wnorm")
            nc.vector.tensor_scalar_mul(wnorm[:, :], expv[:, :], rz[0:1, 0:1])
            nc.sync.dma_start(out=wout_d[b, :], in_=wnorm[0:1, :])

            # wT[p, t] = wnorm[0, t*128+p] via 16 PE transposes
            wt_ps = ps_t.tile([128, 512], F32, tag="pt")
            for t in range(SB):
                nc.tensor.matmul(
                    wt_ps[:, t:t + 1],
                    wnorm[0:1, t * 128:(t + 1) * 128],
                    ident[0:1, 0:1],
                    is_transpose=True, start=True, stop=True,
                )
            wT = sm_pool.tile([128, SB], F32R, tag="wT")
            nc.vector.tensor_copy(wT[:, :], wt_ps[:, 0:SB])

            # D: ctx = wnorm @ keys (fp32; keys streamed again)
            cps0 = ps_sc.tile([1, 512], F32, tag="scps")
            cps1 = ps_sc.tile([1, 512], F32, tag="scps")
            for t in range(SB):
                kg2 = stream.tile([128, H], F32, tag="keys2")
                nc.sync.dma_start(
                    out=kg2[:, :], in_=keys_d[b, t * 128:(t + 1) * 128, :]
                )
                kg2r = stream.tile([128, H], F32R, tag="keys2r")
                nc.gpsimd.tensor_copy(kg2r[:, :], kg2[:, :])
                nc.tensor.matmul(
                    cps0[:, :], wT[:, t:t + 1], kg2r[:, 0:512],
                    start=(t == 0), stop=(t == SB - 1),
                )
                nc.tensor.matmul(
                    cps1[:, :], wT[:, t:t + 1], kg2r[:, 512:1024],
                    start=(t == 0), stop=(t == SB - 1),
                )
            ctx_sb = sm_pool.tile([1, H], F32, tag="ctx")
            nc.vector.tensor_copy(ctx_sb[0:1, 0:512], cps0[:, :])
            nc.vector.tensor_copy(ctx_sb[0:1, 512:1024], cps1[:, :])
            nc.sync.dma_start(out=ctx_d[b, :], in_=ctx_sb[0:1, :])

    nc.compile()
    return nc


def _get_program():
    if "nc" not in _cache:
        _cache["nc"] = _build_program()
    return _cache["nc"]


def _make_in_maps(inputs):
    query = np.asarray(inputs["query"], np.float32).reshape(B, H)
    keys = np.ascontiguousarray(np.asarray(inputs["keys"], np.float32))
    wa = np.ascontiguousarray(np.asarray(inputs["Wa_w"], np.float32))
    ua = np.ascontiguousarray(np.asarray(inputs["Ua_w"], np.float32))
    wab = np.asarray(inputs["Wa_b"], np.float32).reshape(HB, 128)
    uab = np.asarray(inputs["Ua_b"], np.float32).reshape(HB, 128)
    va = np.asarray(inputs["Va_w"], np.float32).reshape(HB, 128)

    in_maps = []
    for c in range(NCORES):
        sl = slice(c * BPC, (c + 1) * BPC)
        in_maps.append({
            "keys": np.ascontiguousarray(keys[sl]),
            "query": np.ascontiguousarray(query[sl]),
            "wa": wa, "ua": ua, "wab": wab, "uab": uab, "va": va,
        })
    return in_maps


def _run(inputs, trace=False, trace_cores=None):
    from concourse.bass_utils import run_bass_kernel_spmd

    nc = _get_program()
    in_maps = _make_in_maps(inputs)
    kw = {}
    if trace:
        kw = {"trace": True, "trace_cores": trace_cores or [0]}
    res = run_bass_kernel_spmd(nc, in_maps, core_ids=list(range(NCORES)), **kw)
    ctx = np.concatenate([r["ctx"] for r in res.results], 0).reshape(B, 1, H)
    w = np.concatenate([r["wout"] for r in res.results], 0).reshape(B, 1, S)
    return (ctx, w), res


def kernel(**inputs):
    (ctx, w), _ = _run(inputs)
    return ctx, w


# revision 18
# speedup vs baseline: 98525.1910x; 98525.1910x over previous
"""Bahdanau attention kernel for 8 Trainium2 NeuronCores.

Data-parallel over batch B=32 (4 batches per core); Wa/Ua/Va weights
replicated on every core. Per core:
  q  = query @ Wa^T + Wa_b + Ua_b          (tiny, on PE)
  kT = transpose(keys) blocks on PE; k = keys @ Ua^T via fp32r matmuls
  h  = tanh(k + q) fused on ScalarE (per-partition bias)
  s  = Va . h via fp32r matmuls -> softmax (Va_b cancels in softmax)
  ctx = softmax(s) @ keys via fp32 matmuls (keys streamed a 2nd time)
"""
import sys

sys.path.insert(0, "/opt/trn_rl_repo")

import numpy as np

B, S, H = 32, 2048, 1024
NCORES = 8
BPC = B // NCORES   # batches per core
HB = H // 128       # 8 blocks of 128 along H
SB = S // 128       # 16 blocks of 128 along S
SCH = 512           # s-chunk for the Ua matmul / tanh / Va pipeline
NSCH = S // SCH
SG = 512            # s-group per keys DMA during the transpose phase

_cache = {}


def _build_program():
    from contextlib import ExitStack

    import concourse.bass as bass  # noqa: F401
    from concourse import bacc, mybir
    from concourse.masks import make_identity
    from concourse.tile import TileContext

    F32 = mybir.dt.float32
    F32R = mybir.dt.float32r
    Act = mybir.ActivationFunctionType

    nc = bacc.Bacc()
    keys_d = nc.declare_dram_parameter("keys", [BPC, S, H], F32, isOutput=False)
    query_d = nc.declare_dram_parameter("query", [BPC, H], F32, isOutput=False)
    wa_d = nc.declare_dram_parameter("wa", [H, H], F32, isOutput=False)
    ua_d = nc.declare_dram_parameter("ua", [H, H], F32, isOutput=False)
    wab_d = nc.declare_dram_parameter("wab", [HB, 128], F32, isOutput=False)
    uab_d = nc.declare_dram_parameter("uab", [HB, 128], F32, isOutput=False)
    va_d = nc.declare_dram_parameter("va", [HB, 128], F32, isOutput=False)
    ctx_d = nc.declare_dram_parameter("ctx", [BPC, H], F32, isOutput=True)
    wout_d = nc.declare_dram_parameter("wout", [BPC, S], F32, isOutput=True)

    with TileContext(nc) as tc, ExitStack() as ex:
        singles = ex.enter_context(tc.tile_pool(name="singles", bufs=1))
        stream = ex.enter_context(tc.tile_pool(name="stream", bufs=3))
        k2r_pool = ex.enter_context(tc.tile_pool(name="k2r", bufs=8))
        ht_pool = ex.enter_context(tc.tile_pool(name="ht", bufs=3))
        keys1_pool = ex.enter_context(tc.tile_pool(name="keys1p", bufs=2))
        sm_pool = ex.enter_context(tc.tile_pool(name="sm", bufs=1))
        ps_t = ex.enter_context(tc.tile_pool(name="ps_t", bufs=3, space="PSUM"))
        ps_mm = ex.enter_context(tc.tile_pool(name="ps_mm", bufs=3, space="PSUM"))
        ps_sc = ex.enter_context(tc.tile_pool(name="ps_sc", bufs=2, space="PSUM"))

        ident = singles.tile([128, 128], F32)
        make_identity(nc, ident[:, :])

        # biases: bias_sum[p, i] = Wa_b[i*128+p] + Ua_b[i*128+p]
        bias_sum = singles.tile([128, HB], F32)
        tmp_b = singles.tile([128, HB], F32)
        nc.scalar.dma_start(out=bias_sum[:, :], in_=wab_d[:, :].rearrange("j p -> p j"))
        nc.scalar.dma_start(out=tmp_b[:, :], in_=uab_d[:, :].rearrange("j p -> p j"))
        nc.vector.tensor_add(bias_sum[:, :], bias_sum[:, :], tmp_b[:, :])

        # vaT[p, i] = Va_w[0, i*128+p], rounded to fp32r
        va_f32 = singles.tile([128, HB], F32)
        nc.scalar.dma_start(out=va_f32[:, :], in_=va_d[:, :].rearrange("j p -> p j"))
        vaT = singles.tile([128, HB], F32R)
        nc.vector.tensor_copy(vaT[:, :], va_f32[:, :])

        def load_T(w_dram, name, pool):
            """DRAM [H,H] row-major W[o,h] -> SBUF [128, HB, H] fp32r W^T:
            wt[p, j, o] = W[o, j*128+p]. 1MB DMA chunks (2 o-blocks each)
            so PE transposes start early and DMA queues parallelize."""
            wt = pool.tile([128, HB, H], F32R, tag=name)
            for i2 in range(0, HB, 2):
                nat = keys1_pool.tile([128, 2, H], F32, tag="keys1", name="nat")
                nc.sync.dma_start(
                    out=nat[:, :, :],
                    in_=w_dram[i2 * 128:(i2 + 2) * 128, :].rearrange(
                        "(o p) h -> p o h", p=128
                    ),
                )
                for oi in range(2):
                    i = i2 + oi
                    for j4 in range(0, HB, 4):
                        pt = ps_t.tile([128, 512], F32, tag="pt")
                        for q in range(4):
                            j = j4 + q
                            nc.tensor.transpose(
                                pt[:, q * 128:(q + 1) * 128],
                                nat[:, oi, j * 128:(j + 1) * 128],
                                ident[:, :],
                            )
                        dst = wt[:, j4:j4 + 4, i * 128:(i + 1) * 128]
                        src = pt[:, :].rearrange("p (q x) -> p q x", q=4)
                        if (i + j4 // 4) % 2 == 0:
                            nc.vector.tensor_copy(dst, src)
                        else:
                            nc.scalar.activation(
                                out=dst, in_=src, func=Act.Copy, scale=1.0
                            )
            return wt

        # ---- q phase: qq[p, i, b] = (query @ Wa^T)[b, i*128+p] + bias_sum
        waT_cm = tc.tile_pool(name="waTp", bufs=1)
        waTp = waT_cm.__enter__()
        waT = load_T(wa_d, "waT", waTp)
        qsb = singles.tile([BPC, H], F32)
        nc.scalar.dma_start(out=qsb[:, :], in_=query_d[:, :])
        qT = singles.tile([128, HB, BPC], F32R)
        for j in range(HB):
            ptq = ps_t.tile([128, 512], F32, tag="pt")
            nc.tensor.transpose(
                ptq[:, 0:BPC], qsb[0:BPC, j * 128:(j + 1) * 128], ident[0:BPC, 0:BPC]
            )
            nc.vector.tensor_copy(qT[:, j, :], ptq[:, 0:BPC])
        qq = singles.tile([128, HB, BPC], F32)
        for i in range(HB):
            pq = ps_mm.tile([128, 512], F32, tag="kpsum")
            for j in range(HB):
                nc.tensor.matmul(
                    pq[:, 0:BPC],
                    waT[:, j, i * 128:(i + 1) * 128],
                    qT[:, j, :],
                    start=(j == 0),
                    stop=(j == HB - 1),
                )
            nc.vector.tensor_scalar_add(qq[:, i, :], pq[:, 0:BPC], bias_sum[:, i:i + 1])

        waT_cm.__exit__(None, None, None)
        kt_pool = ex.enter_context(tc.tile_pool(name="kTp", bufs=1))

        # ---- per-batch pipeline, software-pipelined:
        # A_b = keys transposes, B_b = Ua matmul + tanh + Va + softmax,
        # ctx_b (wT transposes + keys re-stream fp32r matmuls) is sliced and
        # interleaved into A_{b+1} so PE keeps transpose work while the
        # softmax chain and the fp32r key conversions drain.
        wnorms = {}
        CONV_ENG = [0, 0, 1, 2]  # gpsimd, gpsimd, vector, scalar

        def build_ctx_slices(b):
            """Return a list of emitters; phase_A pumps them between groups."""
            wnorm, rz = wnorms.pop(b)
            state = {"kg2r": []}

            def emit_converts():
                # stage + convert all key tiles; buffer depth covers the gap
                # between conversion rate and PE consumption
                if state["kg2r"]:
                    return
                for t in range(SB):
                    kg2 = stream.tile([128, H], F32, tag="keys2", name="kg2")
                    nc.sync.dma_start(
                        out=kg2[:, :], in_=keys_d[b, t * 128:(t + 1) * 128, :]
                    )
                    kg2r = k2r_pool.tile([128, H], F32R, tag="keys2r", name="kg2r")
                    eng = CONV_ENG[t % 4]
                    if eng == 0:
                        nc.gpsimd.tensor_copy(kg2r[:, :], kg2[:, :])
                    elif eng == 1:
                        nc.vector.tensor_copy(kg2r[:, :], kg2[:, :])
                    else:
                        nc.scalar.activation(
                            out=kg2r[:, :], in_=kg2[:, :], func=Act.Copy, scale=1.0
                        )
                    state["kg2r"].append(kg2r)
            state["emit_converts"] = emit_converts

            def s_wt():
                wt_ps = ps_t.tile([128, 512], F32, tag="pt")
                for t in range(SB):
                    nc.tensor.matmul(
                        wt_ps[:, t:t + 1],
                        wnorm[0:1, t * 128:(t + 1) * 128],
                        ident[0:1, 0:1],
                        is_transpose=True, start=True, stop=True,
                    )
                wT = sm_pool.tile([128, SB], F32R, tag="wT", name="wT")
                nc.vector.tensor_copy(wT[:, :], wt_ps[:, 0:SB])
                state["wT"] = wT
                state["cps0"] = ps_sc.tile([1, 512], F32, tag="scps", name="cps0")
                state["cps1"] = ps_sc.tile([1, 512], F32, tag="scps", name="cps1")
                emit_converts()

            def mk_mm(t0, n):
                def f():
                    for t in range(t0, t0 + n):
                        kg2r = state["kg2r"][t]
                        nc.tensor.matmul(
                            state["cps0"][:, :], state["wT"][:, t:t + 1],
                            kg2r[:, 0:512],
                            start=(t == 0), stop=(t == SB - 1),
                        )
                        nc.tensor.matmul(
                            state["cps1"][:, :], state["wT"][:, t:t + 1],
                            kg2r[:, 512:1024],
                            start=(t == 0), stop=(t == SB - 1),
                        )
                return f

            def s_out():
                ctx_sb = sm_pool.tile([1, H], F32, tag="ctx")
                nc.vector.tensor_scalar_mul(
                    ctx_sb[0:1, 0:512], state["cps0"][:, :], rz[0:1, 0:1]
                )
                nc.vector.tensor_scalar_mul(
                    ctx_sb[0:1, 512:1024], state["cps1"][:, :], rz[0:1, 0:1]
                )
                nc.sync.dma_start(out=ctx_d[b, :], in_=ctx_sb[0:1, :])

            slices = [s_wt, mk_mm(0, 4), mk_mm(4, 4), mk_mm(8, 4),
                      lambda: (mk_mm(12, 4)(), s_out())]
            slices[0].emit_converts = emit_converts  # type: ignore[attr-defined]
            return slices

        def phase_A(b, ctx_slices=None, kT=None, groups=None):
            pump = list(ctx_slices or [])
            if kT is None:
                kT = kt_pool.tile([128, HB, S], F32R, tag="kT", name="kT")
            for g in groups if groups is not None else range(S // SG):
                kg = keys1_pool.tile([128, 4, H], F32, tag="keys1", name="kg")
                nc.sync.dma_start(
                    out=kg[:, :, :],
                    in_=keys_d[b, g * SG:(g + 1) * SG, :].rearrange(
                        "(t p) h -> p t h", p=128
                    ),
                )
                for j in range(HB):
                    pt = ps_t.tile([128, 512], F32, tag="pt")
                    for t4 in range(SG // 128):
                        nc.tensor.transpose(
                            pt[:, t4 * 128:(t4 + 1) * 128],
                            kg[:, t4, j * 128:(j + 1) * 128],
                            ident[:, :],
                        )
                    dst = kT[:, j, g * SG:(g + 1) * SG]
                    if j % 2 == 0:
                        nc.vector.tensor_copy(dst, pt[:, :])
                    else:
                        nc.scalar.activation(
                            out=dst, in_=pt[:, :], func=Act.Copy, scale=1.0
                        )
                if g >= 1 and pump:
                    pump.pop(0)()
            while pump:
                pump.pop(0)()
            return kT

        def phase_B(b, kT):
            scores = sm_pool.tile([1, S], F32, tag="scores")
            pending = None
            scps = []
            for c in range(NSCH):
                scp = ps_sc.tile([1, SCH], F32, tag="scps")
                scps.append(scp)
                for i in range(HB):
                    kp = ps_mm.tile([128, SCH], F32, tag="kpsum")
                    for j in range(HB):
                        nc.tensor.matmul(
                            kp[:, :],
                            uaT[:, j, i * 128:(i + 1) * 128],
                            kT[:, j, c * SCH:(c + 1) * SCH],
                            start=(j == 0),
                            stop=(j == HB - 1),
                        )
                    ht = ht_pool.tile([128, SCH], F32R, tag="ht")
                    nc.scalar.activation(
                        out=ht[:, :], in_=kp[:, :], func=Act.Tanh,
                        bias=qq[:, i, b:b + 1], scale=1.0,
                    )
                    if pending is not None:
                        pht, pscp, pi = pending
                        nc.tensor.matmul(
                            pscp[:, :], vaT[:, pi:pi + 1], pht[:, :],
                            start=(pi == 0), stop=(pi == HB - 1),
                        )
                        if pi == HB - 1:
                            pc = scps.index(pscp)
                            nc.vector.tensor_copy(
                                scores[0:1, pc * SCH:(pc + 1) * SCH], pscp[:, :]
                            )
                    pending = (ht, scp, i)
            pht, pscp, pi = pending
            nc.tensor.matmul(
                pscp[:, :], vaT[:, pi:pi + 1], pht[:, :],
                start=(pi == 0), stop=(pi == HB - 1),
            )
            nc.vector.tensor_copy(
                scores[0:1, (NSCH - 1) * SCH:NSCH * SCH], pscp[:, :]
            )

            # softmax on partition 0 (ACT/DVE; overlaps A_{b+1} on PE)
            nmax = sm_pool.tile([1, 1], F32, tag="nmax")
            nc.vector.tensor_reduce(
                out=nmax[:, :], in_=scores[:, :],
                axis=mybir.AxisListType.X, op=mybir.AluOpType.max, negate=True,
            )
            expv = sm_pool.tile([1, S], F32, tag="expv")
            ssum = sm_pool.tile([1, 1], F32, tag="ssum")
            nc.scalar.activation(
                out=expv[:, :], in_=scores[:, :], func=Act.Exp,
                bias=nmax[:, :], scale=1.0, accum_out=ssum[:, :],
            )
            rz = sm_pool.tile([1, 1], F32, tag="rz")
            nc.vector.reciprocal(rz[:, :], ssum[:, :])
            # normalized weights overwrite the scores tile (saves SBUF);
            # ctx uses the raw exp + folds 1/Z into its PSUM drain, so the
            # normalize is off the PE critical path
            nc.vector.tensor_scalar_mul(scores[:, :], expv[:, :], rz[0:1, 0:1])
            nc.sync.dma_start(out=wout_d[b, :], in_=scores[0:1, :])
            wnorms[b] = (expv, rz)

        kT0 = kt_pool.tile([128, HB, S], F32R, tag="kT", name="kT0")
        phase_A(0, kT=kT0, groups=[0, 1])
        uaT = load_T(ua_d, "uaT", singles)
        phase_A(0, kT=kT0, groups=[2, 3])
        phase_B(0, kT0)
        for b in range(1, BPC):
            last = b == BPC - 1
            slices = build_ctx_slices(b - 1)
            kT = phase_A(b, slices)
            phase_B(b, kT)
            if last:
                tail = build_ctx_slices(b)
                tail[0].emit_converts()  # stage+convert during B's tail
        for sl in tail:
            sl()

    nc.compile()
    return nc


def _get_program():
    if "nc" not in _cache:
        _cache["nc"] = _build_program()
    return _cache["nc"]


def _make_in_maps(inputs):
    query = np.asarray(inputs["query"], np.float32).reshape(B, H)
    keys = np.ascontiguousarray(np.asarray(inputs["keys"], np.float32))
    wa = np.ascontiguousarray(np.asarray(inputs["Wa_w"], np.float32))
    ua = np.ascontiguousarray(np.asarray(inputs["Ua_w"], np.float32))
    wab = np.asarray(inputs["Wa_b"], np.float32).reshape(HB, 128)
    uab = np.asarray(inputs["Ua_b"], np.float32).reshape(HB, 128)
    va = np.asarray(inputs["Va_w"], np.float32).reshape(HB, 128)

    in_maps = []
    for c in range(NCORES):
        sl = slice(c * BPC, (c + 1) * BPC)
        in_maps.append({
            "keys": np.ascontiguousarray(keys[sl]),
            "query": np.ascontiguousarray(query[sl]),
            "wa": wa, "ua": ua, "wab": wab, "uab": uab, "va": va,
        })
    return in_maps


def _run(inputs, trace=False, trace_cores=None):
    from concourse.bass_utils import run_bass_kernel_spmd

    nc = _get_program()
    in_maps = _make_in_maps(inputs)
    kw = {}
    if trace:
        kw = {"trace": True, "trace_cores": trace_cores or [0]}
    res = run_bass_kernel_spmd(nc, in_maps, core_ids=list(range(NCORES)), **kw)
    ctx = np.concatenate([r["ctx"] for r in res.results], 0).reshape(B, 1, H)
    w = np.concatenate([r["wout"] for r in res.results], 0).reshape(B, 1, S)
    return (ctx, w), res


def kernel(**inputs):
    (ctx, w), _ = _run(inputs)
    return ctx, w
